# revision 20
# baseline (speedup 1.0000x reference)
"""Trainium2 Bass kernel for nn_AttentionHead_Hybrid1 (quantum-inspired attention head).

Computation (per batch b of a [B=64, S=1024, D=64] input):
    n_i   = ||x_i||;  u_i = x_i / n_i
    W     = givens_orthogonal(phi)                (tiny, sequential -> host)
    A     = (n_i n_j) (u_i^T W^T u_j)^2           (S x S scores)
    V     = x Wv^T + bv
    out   = LayerNorm(softmax(A/sqrt(D)) V + x)

Kernel strategy (data-parallel over batch, 8 batches per NeuronCore):
  * Fold norms into the score matmul:  ut_i = x_i / sqrt(n_i),
    G^T = (ut W) ut^T, so A^T = G^T * G^T elementwise.
  * Scores computed directly in transposed [j, i] layout so the PV matmul
    needs no transpose of the softmax matrix.
  * Softmax denominator comes out of the PV matmul via an appended
    all-ones column of V; the sqrt(n_j) row-rescale of V is folded into the
    per-partition BIAS of the exp activation (exp(G^2/8 + ln sqrt(n_j))),
    making V' = ut Wv_aug^T directly usable and column 64 exactly 1/sqrt(n)
    whose rescale lands on the softmax denominator consistently.
  * No max-subtraction needed: max exponent of A/sqrt(D) is ~4.
  * The softmax division is eliminated entirely: LayerNorm is invariant to
    per-row scaling, so we feed it h' = (P V) + denom * x and scale the
    variance epsilon by denom^2.
  * 1/sqrt and x^(+-1/4) are computed as exp(c*ln(x)); the activation-table
    map is restricted to natural_log_exp_and_others so there is exactly one
    ACT table load in the whole kernel.
  * The whole matmul path runs in BF16 (measured end-to-end error ~1e-4).
  * Elementwise work is fused into few wide ops (free-dim-broadcast APs for
    per-row scalars) because per-instruction overhead dominates DVE/ACT.
"""

import math
import sys

import numpy as np

sys.path.insert(0, "/opt/trn_rl_repo")

import concourse.bass as bass
import concourse.bacc as bacc
import concourse.tile as tile
from concourse import mybir
from concourse.bass_utils import run_bass_kernel_spmd

try:
    import ml_dtypes
    BF16_NP = ml_dtypes.bfloat16
except ImportError:  # pragma: no cover
    BF16_NP = None

F32 = mybir.dt.float32
BF16 = mybir.dt.bfloat16
AX = mybir.AxisListType.X
OP = mybir.AluOpType
AF = mybir.ActivationFunctionType

B, S, D = 64, 1024, 64
NCORES = 8
NB = B // NCORES          # batches per core
NC = S // 128             # 128-row chunks per batch
DA = D + 1                # V augmented with ones column
INV_SQRT_D = 1.0 / math.sqrt(D)
LN_EPS = 1e-5

# score chunk pairs whose squaring runs on VectorE instead of ScalarE
DVE_SQUARE_PAIRS = frozenset({0, 2})  # pairs: 0->(0,1) 1->(2,3) 2->(4,5) 3->(6,7)

_ACT_SET = "natural_log_exp_and_others"


def _patch_act_tables():
    """Make every activation resolve to one table set (it contains every
    function this kernel uses), so the compiled stream has exactly one
    ACT_TABLE_LOAD instead of ping-ponging between per-anchor sets."""
    from concourse import hw_specs

    if getattr(bacc, "_act_tables_patched", False):
        return
    orig = hw_specs.get_activation_tables

    def patched(arch):
        tabs = orig(arch)
        return {
            name: (funcs if name == _ACT_SET else set())
            for name, funcs in tabs.items()
        }

    bacc.get_activation_tables = patched
    bacc._act_tables_patched = True


def _givens_orthogonal(phi: np.ndarray, d: int) -> np.ndarray:
    pairs = [(i, i + 1) for i in range(d - 1)] + [(i, i + 1) for i in range(d - 3, -1, -1)]
    W = np.eye(d, dtype=np.float64)
    p = phi.astype(np.float64)
    for k, (i, j) in enumerate(pairs):
        c, s = np.cos(p[k]), np.sin(p[k])
        ri, rj = W[i].copy(), W[j].copy()
        W[i] = c * ri + s * rj
        W[j] = -s * ri + c * rj
    return W.astype(np.float32)


def _bcast_inner(ap, n):
    """[P, NC] -> [P, NC, n] with stride-0 inner dim."""
    return ap.unsqueeze(2).broadcast_to((ap.shape[0], ap.shape[1], n))


def _build_nc() -> bass.Bass:
    _patch_act_tables()
    nc = bacc.Bacc("TRN2", target_bir_lowering=False, debug=False, num_devices=NCORES)

    x_d = nc.dram_tensor("x", [NB, S, D], F32, kind="ExternalInput").ap()
    w_d = nc.dram_tensor("wg", [D, D], BF16, kind="ExternalInput").ap()
    wv_d = nc.dram_tensor("wv_aug", [DA, DA], BF16, kind="ExternalInput").ap()
    idb_d = nc.dram_tensor("ident_b", [128, 128], BF16, kind="ExternalInput").ap()
    idf_d = nc.dram_tensor("ident_f", [128, 128], F32, kind="ExternalInput").ap()
    out_d = nc.dram_tensor("out", [NB, S, D], F32, kind="ExternalOutput").ap()

    with tile.TileContext(nc) as tc:
        with (
            tc.tile_pool(name="const", bufs=1) as constp,
            tc.tile_pool(name="xin", bufs=3) as xin,
            tc.tile_pool(name="prep", bufs=2) as prep,
            tc.tile_pool(name="stats", bufs=2) as stats,
            tc.tile_pool(name="big", bufs=3) as big,
            tc.tile_pool(name="score", bufs=4) as score,
            tc.tile_pool(name="lnp", bufs=3) as lnp,
            tc.tile_pool(name="pbig", bufs=3, space="PSUM") as pbig,
            tc.tile_pool(name="po", bufs=1, space="PSUM") as pop,
        ):
            w_sb = constp.tile([D, D], BF16)
            nc.sync.dma_start(w_sb, w_d)
            wv_sb = constp.tile([DA, DA], BF16)
            nc.sync.dma_start(wv_sb, wv_d)
            ident_b = constp.tile([128, 128], BF16)
            nc.sync.dma_start(ident_b, idb_d)
            ident_f = constp.tile([128, 128], F32)
            nc.sync.dma_start(ident_f, idf_d)

            for b in range(NB):
                # ---- load x[b] as [128, NC, 64] --------------------------------
                x_sb = xin.tile([128, NC, D], F32)
                nc.sync.dma_start(x_sb, x_d[b].rearrange("(c p) d -> p c d", p=128))

                # ---- norms: nsq; s = nsq^-1/4; lnq = ln(sqrt n) ----------------
                xsq = prep.tile([128, NC, D], F32, tag="xsq")
                nc.scalar.activation(xsq, x_sb, AF.Square)
                nsq = stats.tile([128, NC], F32, tag="nsq")
                nc.vector.reduce_sum(nsq, xsq, axis=AX)
                lnn = stats.tile([128, NC], F32, tag="lnn")
                nc.scalar.activation(lnn, nsq, AF.Ln)
                s_t = stats.tile([128, NC], F32, tag="s")
                nc.scalar.activation(s_t, lnn, AF.Exp, scale=-0.25)
                # exp bias: ln(sqrt n_j) = 0.25 * ln(nsq)
                lnq = stats.tile([128, NC], F32, tag="lnq")
                nc.vector.tensor_scalar(
                    out=lnq, in0=lnn, scalar1=0.25, scalar2=None, op0=OP.mult,
                )

                # ---- ut rows (bf16): [128, NC, 65]; col 64 = s -----------------
                ut = prep.tile([128, NC, DA], BF16, tag="ut")
                nc.vector.tensor_mul(
                    ut[:, :, 0:D], x_sb, _bcast_inner(s_t, D)
                )
                nc.vector.tensor_copy(ut[:, :, D], s_t)

                # ---- transpose ut chunks into ONE psum bank, 1 copy out --------
                ptall = pbig.tile([DA, S], BF16, tag="g")
                for c in range(NC):
                    nc.tensor.transpose(ptall[:, c * 128:(c + 1) * 128], ut[:, c, :], ident_b)
                utT = big.tile([DA, S], BF16, tag="utT")
                nc.vector.tensor_copy(utT, ptall)

                # ---- Z^T = W^T UT  [64, 1024] bf16 -----------------------------
                zpall = pbig.tile([D, S], F32, tag="g")
                for h in range(2):
                    nc.tensor.matmul(
                        zpall[:, h * 512:(h + 1) * 512], w_sb[0:D, :],
                        utT[0:D, h * 512:(h + 1) * 512], start=True, stop=True,
                    )
                zT = big.tile([D, S], BF16, tag="zT")
                nc.vector.tensor_copy(zT, zpall)

                # ---- V'' = ut Wv_aug^T rows (no sqrt(n) rescale needed) --------
                # K = 65: rows 0..63 = Wv^T (zero col 64), row 64 = [bv, 1]
                vpall = pbig.tile([128, NC, 128], F32, tag="g")
                for c in range(NC):
                    nc.tensor.matmul(
                        vpall[:, c, 0:DA], utT[0:DA, c * 128:(c + 1) * 128], wv_sb,
                        start=True, stop=True,
                    )
                v_sb = prep.tile([128, NC, DA], BF16, tag="v")
                nc.vector.tensor_copy(v_sb, vpall[:, :, 0:DA])

                # ---- scores + softmax numerators + PV, 2 chunks per wave ------
                po0 = pop.tile([DA, 512], F32, tag="po0")
                po1 = pop.tile([DA, 512], F32, tag="po1")
                for w in range(NC // 2):
                    gps = []
                    for jc in (2 * w, 2 * w + 1):
                        gp = pbig.tile([128, S], F32, tag="g")
                        for h in range(2):
                            nc.tensor.matmul(
                                gp[:, h * 512:(h + 1) * 512],
                                zT[:, jc * 128:(jc + 1) * 128],
                                utT[0:D, h * 512:(h + 1) * 512],
                                start=True, stop=True,
                            )
                        gps.append(gp)
                    sq = score.tile([128, 2, S], BF16, tag="sq")
                    if w in DVE_SQUARE_PAIRS:
                        gc = score.tile([128, 2, S], BF16, tag="gc")
                        nc.vector.tensor_copy(gc[:, 0, :], gps[0])
                        nc.vector.tensor_copy(gc[:, 1, :], gps[1])
                        nc.vector.tensor_mul(sq, gc, gc)
                    else:
                        nc.scalar.activation(sq[:, 0, :], gps[0], AF.Square)
                        nc.scalar.activation(sq[:, 1, :], gps[1], AF.Square)
                    p_t = score.tile([128, 2, S], BF16, tag="p")
                    for k, jc in enumerate((2 * w, 2 * w + 1)):
                        nc.scalar.activation(
                            p_t[:, k, :], sq[:, k, :], AF.Exp,
                            scale=INV_SQRT_D, bias=lnq[:, jc:jc + 1],
                        )
                        nc.tensor.matmul(
                            po0, v_sb[:, jc, :], p_t[:, k, 0:512],
                            start=(jc == 0), stop=(jc == NC - 1),
                        )
                        nc.tensor.matmul(
                            po1, v_sb[:, jc, :], p_t[:, k, 512:1024],
                            start=(jc == 0), stop=(jc == NC - 1),
                        )

                # ---- O = [PV | denom]^T  [65, 1024] fp32 -----------------------
                o_sb = big.tile([DA, S], F32, tag="o")
                nc.vector.tensor_copy(o_sb[:, 0:512], po0)
                nc.scalar.copy(o_sb[:, 512:1024], po1)

                # ---- transpose O chunks into one psum region -------------------
                htall = pbig.tile([128, NC, 128], F32, tag="g")
                for c in range(NC):
                    nc.tensor.transpose(
                        htall[:, c, 0:DA], o_sb[:, c * 128:(c + 1) * 128],
                        ident_f[0:DA, 0:DA],
                    )
                # denom row per chunk -> SBUF
                dn = lnp.tile([128, NC], F32, tag="dn")
                nc.vector.tensor_copy(dn, htall[:, :, D])
                # h' = denom * x + attn_numer  (LayerNorm scale-invariance)
                xd = lnp.tile([128, NC, D], F32, tag="xd")
                nc.vector.tensor_mul(xd, x_sb, _bcast_inner(dn, D))
                hp = lnp.tile([128, NC, D], F32, tag="hp")
                nc.vector.tensor_add(hp, xd, htall[:, :, 0:D])

                # ---- LayerNorm over D (reduce-based mean/var) ------------------
                hsq = lnp.tile([128, NC, D], F32, tag="hsq")
                nc.vector.tensor_mul(hsq, hp, hp)
                sums = lnp.tile([128, NC], F32, tag="sums")
                nc.vector.reduce_sum(sums, hp, axis=AX)
                sumsq = lnp.tile([128, NC], F32, tag="sumsq")
                nc.vector.reduce_sum(sumsq, hsq, axis=AX)
                mean = lnp.tile([128, NC], F32, tag="mean")
                nc.vector.tensor_scalar(
                    out=mean, in0=sums, scalar1=1.0 / D, scalar2=None, op0=OP.mult,
                )
                msq = lnp.tile([128, NC], F32, tag="msq")
                nc.vector.tensor_mul(msq, mean, mean)
                # var = sumsq/D - mean^2
                var = lnp.tile([128, NC], F32, tag="var")
                nc.vector.scalar_tensor_tensor(
                    out=var, in0=sumsq, scalar=1.0 / D, in1=msq,
                    op0=OP.mult, op1=OP.subtract,
                )
                dn2 = lnp.tile([128, NC], F32, tag="dn2")
                nc.vector.tensor_mul(dn2, dn, dn)
                vpe = stats.tile([128, NC], F32, tag="vpe")
                nc.vector.scalar_tensor_tensor(
                    out=vpe, in0=dn2, scalar=LN_EPS, in1=var,
                    op0=OP.mult, op1=OP.add,
                )
                lnv = stats.tile([128, NC], F32, tag="lnv")
                nc.scalar.activation(lnv, vpe, AF.Ln)
                rstd = stats.tile([128, NC], F32, tag="rstd")
                nc.scalar.activation(rstd, lnv, AF.Exp, scale=-0.5)

                # ---- (h' - mean) * rstd, broadcast ops -------------------------
                hm = lnp.tile([128, NC, D], F32, tag="hm")
                nc.vector.tensor_tensor(
                    out=hm, in0=hp, in1=_bcast_inner(mean, D),
                    op=OP.subtract,
                )
                o_rows = lnp.tile([128, NC, D], F32, tag="orows")
                nc.vector.tensor_mul(o_rows, hm, _bcast_inner(rstd, D))
                nc.sync.dma_start(
                    out_d[b].rearrange("(c p) d -> p c d", p=128), o_rows
                )
    nc.compile()
    return nc


_CACHED = None


def _get_nc():
    global _CACHED
    if _CACHED is None:
        _CACHED = _build_nc()
    return _CACHED


def _to_bf16(a: np.ndarray) -> np.ndarray:
    if BF16_NP is not None:
        return a.astype(BF16_NP)
    u = np.ascontiguousarray(a.astype(np.float32)).view(np.uint32)
    r = ((u >> 16) & 1).astype(np.uint32)
    return (((u + 0x7FFF + r) >> 16).astype(np.uint16)).view(np.uint16)


def kernel(x: np.ndarray, Wv: np.ndarray, bv: np.ndarray, phi: np.ndarray) -> np.ndarray:
    x = np.ascontiguousarray(np.asarray(x, np.float32))
    Wv = np.asarray(Wv, np.float32)
    bv = np.asarray(bv, np.float32)
    phi = np.asarray(phi, np.float32)
    assert x.shape == (B, S, D), x.shape

    wg = _givens_orthogonal(phi, D)
    wv_aug = np.zeros((DA, DA), np.float32)
    wv_aug[0:D, 0:D] = Wv.T
    wv_aug[D, 0:D] = bv
    wv_aug[D, D] = 1.0

    nc = _get_nc()
    in_maps = [
        {
            "x": np.ascontiguousarray(x[c * NB:(c + 1) * NB]),
            "wg": _to_bf16(wg),
            "wv_aug": _to_bf16(wv_aug),
            "ident_b": _to_bf16(np.eye(128, dtype=np.float32)),
            "ident_f": np.eye(128, dtype=np.float32),
        }
        for c in range(NCORES)
    ]
    res = run_bass_kernel_spmd(nc, in_maps, list(range(NCORES)))
    out = np.concatenate([res.results[c]["out"] for c in range(NCORES)], axis=0)
    return out.astype(np.float32)


if __name__ == "__main__":
    rng = np.random.default_rng(0)
    x = rng.standard_normal((B, S, D)).astype(np.float32)
    Wv = (rng.standard_normal((D, D)) / math.sqrt(D)).astype(np.float32)
    bv = (rng.standard_normal(D) * 0.01).astype(np.float32)
    phi = rng.uniform(0, 2 * math.pi, 2 * D - 3).astype(np.float32)
    y = kernel(x=x, Wv=Wv, bv=bv, phi=phi)
    print("out", y.shape, y.dtype, np.abs(y).mean())


# revision 22
# speedup vs baseline: 1.1716x; 1.1716x over previous
"""Trainium2 Bass kernel for nn_AttentionHead_Hybrid1 (quantum-inspired attention head).

Computation (per batch b of a [B=64, S=1024, D=64] input):
    n_i   = ||x_i||;  u_i = x_i / n_i
    W     = givens_orthogonal(phi)                (tiny, sequential -> host)
    A     = (n_i n_j) (u_i^T W^T u_j)^2           (S x S scores)
    V     = x Wv^T + bv
    out   = LayerNorm(softmax(A/sqrt(D)) V + x)

Kernel strategy (data-parallel over batch, 8 batches per NeuronCore):
  * Fold norms into the score matmul:  ut_i = x_i / sqrt(n_i),
    G^T = (ut W) ut^T, so A^T = G^T * G^T elementwise.
  * Scores computed directly in transposed [j, i] layout so the PV matmul
    needs no transpose of the softmax matrix.
  * Softmax denominator comes out of the PV matmul via an appended
    all-ones column of V; the sqrt(n_j) row-rescale of V is folded into the
    per-partition BIAS of the exp activation (exp(G^2/8 + ln sqrt(n_j))),
    making V' = ut Wv_aug^T directly usable and column 64 exactly 1/sqrt(n)
    whose rescale lands on the softmax denominator consistently.
  * No max-subtraction needed: max exponent of A/sqrt(D) is ~4.
  * The softmax division is eliminated entirely: LayerNorm is invariant to
    per-row scaling, so we feed it h' = (P V) + denom * x and scale the
    variance epsilon by denom^2.
  * 1/sqrt and x^(+-1/4) are computed as exp(c*ln(x)); the activation-table
    map is restricted to natural_log_exp_and_others so there is exactly one
    ACT table load in the whole kernel.
  * The whole matmul path runs in BF16 (measured end-to-end error ~1e-4).
  * Elementwise work is fused into few wide ops (free-dim-broadcast APs for
    per-row scalars) because per-instruction overhead dominates DVE/ACT.
"""

import math
import sys

import numpy as np

sys.path.insert(0, "/opt/trn_rl_repo")

import concourse.bass as bass
import concourse.bacc as bacc
import concourse.tile as tile
from concourse import mybir
from concourse.bass_utils import run_bass_kernel_spmd

try:
    import ml_dtypes
    BF16_NP = ml_dtypes.bfloat16
except ImportError:  # pragma: no cover
    BF16_NP = None

F32 = mybir.dt.float32
BF16 = mybir.dt.bfloat16
AX = mybir.AxisListType.X
OP = mybir.AluOpType
AF = mybir.ActivationFunctionType

B, S, D = 64, 1024, 64
NCORES = 8
NB = B // NCORES          # batches per core
NC = S // 128             # 128-row chunks per batch
DA = D + 1                # V augmented with ones column
INV_SQRT_D = 1.0 / math.sqrt(D)
LN_EPS = 1e-5

# score chunk pairs whose squaring runs on VectorE instead of ScalarE
DVE_SQUARE_PAIRS = frozenset({0, 2})  # pairs: 0->(0,1) 1->(2,3) 2->(4,5) 3->(6,7)

_ACT_SET = "natural_log_exp_and_others"


def _patch_act_tables():
    """Make every activation resolve to one table set (it contains every
    function this kernel uses), so the compiled stream has exactly one
    ACT_TABLE_LOAD instead of ping-ponging between per-anchor sets."""
    from concourse import hw_specs

    if getattr(bacc, "_act_tables_patched", False):
        return
    orig = hw_specs.get_activation_tables

    def patched(arch):
        tabs = orig(arch)
        return {
            name: (funcs if name == _ACT_SET else set())
            for name, funcs in tabs.items()
        }

    bacc.get_activation_tables = patched
    bacc._act_tables_patched = True


def _givens_orthogonal(phi: np.ndarray, d: int) -> np.ndarray:
    pairs = [(i, i + 1) for i in range(d - 1)] + [(i, i + 1) for i in range(d - 3, -1, -1)]
    W = np.eye(d, dtype=np.float64)
    p = phi.astype(np.float64)
    for k, (i, j) in enumerate(pairs):
        c, s = np.cos(p[k]), np.sin(p[k])
        ri, rj = W[i].copy(), W[j].copy()
        W[i] = c * ri + s * rj
        W[j] = -s * ri + c * rj
    return W.astype(np.float32)


def _bcast_inner(ap, n):
    """[P, NC] -> [P, NC, n] with stride-0 inner dim."""
    return ap.unsqueeze(2).broadcast_to((ap.shape[0], ap.shape[1], n))


def _build_nc() -> bass.Bass:
    _patch_act_tables()
    nc = bacc.Bacc("TRN2", target_bir_lowering=False, debug=False, num_devices=NCORES)

    x_d = nc.dram_tensor("x", [NB, S, D], F32, kind="ExternalInput").ap()
    w_d = nc.dram_tensor("wg", [D, D], BF16, kind="ExternalInput").ap()
    wv_d = nc.dram_tensor("wv_aug", [DA, DA], BF16, kind="ExternalInput").ap()
    idb_d = nc.dram_tensor("ident_b", [128, 128], BF16, kind="ExternalInput").ap()
    idf_d = nc.dram_tensor("ident_f", [128, 128], F32, kind="ExternalInput").ap()
    out_d = nc.dram_tensor("out", [NB, S, D], F32, kind="ExternalOutput").ap()

    with tile.TileContext(nc) as tc:
        with (
            tc.tile_pool(name="const", bufs=1) as constp,
            tc.tile_pool(name="xin", bufs=3) as xin,
            tc.tile_pool(name="prep", bufs=2) as prep,
            tc.tile_pool(name="stats", bufs=2) as stats,
            tc.tile_pool(name="big", bufs=3) as big,
            tc.tile_pool(name="score", bufs=4) as score,
            tc.tile_pool(name="lnp", bufs=3) as lnp,
            tc.tile_pool(name="pbig", bufs=3, space="PSUM") as pbig,
            tc.tile_pool(name="po", bufs=1, space="PSUM") as pop,
        ):
            w_sb = constp.tile([D, D], BF16)
            nc.sync.dma_start(w_sb, w_d)
            wv_sb = constp.tile([DA, DA], BF16)
            nc.sync.dma_start(wv_sb, wv_d)
            ident_b = constp.tile([128, 128], BF16)
            nc.sync.dma_start(ident_b, idb_d)
            ident_f = constp.tile([128, 128], F32)
            nc.sync.dma_start(ident_f, idf_d)

            def emit_prep(b):
                # ---- load x[b] as [128, NC, 64] --------------------------------
                x_sb = xin.tile([128, NC, D], F32)
                nc.sync.dma_start(x_sb, x_d[b].rearrange("(c p) d -> p c d", p=128))

                # ---- norms: nsq; s = nsq^-1/4; lnq = ln(sqrt n) ----------------
                xsq = prep.tile([128, NC, D], F32, tag="xsq")
                nc.scalar.activation(xsq, x_sb, AF.Square)
                nsq = stats.tile([128, NC], F32, tag="nsq")
                nc.vector.reduce_sum(nsq, xsq, axis=AX)
                lnn = stats.tile([128, NC], F32, tag="lnn")
                nc.scalar.activation(lnn, nsq, AF.Ln)
                s_t = stats.tile([128, NC], F32, tag="s")
                nc.scalar.activation(s_t, lnn, AF.Exp, scale=-0.25)
                # exp bias: ln(sqrt n_j) = 0.25 * ln(nsq)
                lnq = stats.tile([128, NC], F32, tag="lnq")
                nc.vector.tensor_scalar(
                    out=lnq, in0=lnn, scalar1=0.25, scalar2=None, op0=OP.mult,
                )

                # ---- ut rows (bf16): [128, NC, 65]; col 64 = s -----------------
                ut = prep.tile([128, NC, DA], BF16, tag="ut")
                nc.vector.tensor_mul(
                    ut[:, :, 0:D], x_sb, _bcast_inner(s_t, D)
                )
                nc.vector.tensor_copy(ut[:, :, D], s_t)

                # ---- transpose ut chunks into ONE psum bank, 1 copy out --------
                ptall = pbig.tile([DA, S], BF16, tag="g")
                for c in range(NC):
                    nc.tensor.transpose(ptall[:, c * 128:(c + 1) * 128], ut[:, c, :], ident_b)
                utT = big.tile([DA, S], BF16, tag="utT")
                nc.vector.tensor_copy(utT, ptall)

                # ---- Z^T = W^T UT  [64, 1024] bf16 -----------------------------
                zpall = pbig.tile([D, S], F32, tag="g")
                for h in range(2):
                    nc.tensor.matmul(
                        zpall[:, h * 512:(h + 1) * 512], w_sb[0:D, :],
                        utT[0:D, h * 512:(h + 1) * 512], start=True, stop=True,
                    )
                zT = big.tile([D, S], BF16, tag="zT")
                nc.vector.tensor_copy(zT, zpall)

                # ---- V'' = ut Wv_aug^T rows (no sqrt(n) rescale needed) --------
                # K = 65: rows 0..63 = Wv^T (zero col 64), row 64 = [bv, 1]
                vpall = pbig.tile([128, NC, 128], F32, tag="g")
                for c in range(NC):
                    nc.tensor.matmul(
                        vpall[:, c, 0:DA], utT[0:DA, c * 128:(c + 1) * 128], wv_sb,
                        start=True, stop=True,
                    )
                v_sb = prep.tile([128, NC, DA], BF16, tag="v")
                nc.vector.tensor_copy(v_sb, vpall[:, :, 0:DA])
                return x_sb, lnq, utT, zT, v_sb

            state = {0: emit_prep(0)}
            for b in range(NB):
                x_sb, lnq, utT, zT, v_sb = state.pop(b)
                # ---- scores + softmax numerators + PV, 2 chunks per wave ------
                po0 = pop.tile([DA, 512], F32, tag="po0")
                po1 = pop.tile([DA, 512], F32, tag="po1")
                for w in range(NC // 2):
                    gps = []
                    for jc in (2 * w, 2 * w + 1):
                        gp = pbig.tile([128, S], F32, tag="g")
                        for h in range(2):
                            nc.tensor.matmul(
                                gp[:, h * 512:(h + 1) * 512],
                                zT[:, jc * 128:(jc + 1) * 128],
                                utT[0:D, h * 512:(h + 1) * 512],
                                start=True, stop=True,
                            )
                        gps.append(gp)
                    sq = score.tile([128, 2, S], BF16, tag="sq")
                    if w in DVE_SQUARE_PAIRS:
                        gc = score.tile([128, 2, S], BF16, tag="gc")
                        nc.vector.tensor_copy(gc[:, 0, :], gps[0])
                        nc.vector.tensor_copy(gc[:, 1, :], gps[1])
                        nc.vector.tensor_mul(sq, gc, gc)
                    else:
                        nc.scalar.activation(sq[:, 0, :], gps[0], AF.Square)
                        nc.scalar.activation(sq[:, 1, :], gps[1], AF.Square)
                    p_t = score.tile([128, 2, S], BF16, tag="p")
                    for k, jc in enumerate((2 * w, 2 * w + 1)):
                        nc.scalar.activation(
                            p_t[:, k, :], sq[:, k, :], AF.Exp,
                            scale=INV_SQRT_D, bias=lnq[:, jc:jc + 1],
                        )
                        nc.tensor.matmul(
                            po0, v_sb[:, jc, :], p_t[:, k, 0:512],
                            start=(jc == 0), stop=(jc == NC - 1),
                        )
                        nc.tensor.matmul(
                            po1, v_sb[:, jc, :], p_t[:, k, 512:1024],
                            start=(jc == 0), stop=(jc == NC - 1),
                        )

                if b + 1 < NB:
                    state[b + 1] = emit_prep(b + 1)

                # ---- O = [PV | denom]^T  [65, 1024] fp32 -----------------------
                o_sb = big.tile([DA, S], F32, tag="o")
                nc.vector.tensor_copy(o_sb[:, 0:512], po0)
                nc.scalar.copy(o_sb[:, 512:1024], po1)

                # ---- transpose O chunks into one psum region -------------------
                htall = pbig.tile([128, NC, 128], F32, tag="g")
                for c in range(NC):
                    nc.tensor.transpose(
                        htall[:, c, 0:DA], o_sb[:, c * 128:(c + 1) * 128],
                        ident_f[0:DA, 0:DA],
                    )
                # denom row per chunk -> SBUF
                dn = lnp.tile([128, NC], F32, tag="dn")
                nc.vector.tensor_copy(dn, htall[:, :, D])
                # h' = denom * x + attn_numer  (LayerNorm scale-invariance)
                xd = lnp.tile([128, NC, D], F32, tag="xd")
                nc.vector.tensor_mul(xd, x_sb, _bcast_inner(dn, D))
                hp = lnp.tile([128, NC, D], F32, tag="hp")
                nc.vector.tensor_add(hp, xd, htall[:, :, 0:D])

                # ---- LayerNorm over D ------------------------------------------
                bst = lnp.tile([128, NC, 6], F32, tag="bst")
                mv = lnp.tile([128, NC, 2], F32, tag="mv")
                for c in range(NC):
                    nc.vector.bn_stats(bst[:, c, :], hp[:, c, :])
                    nc.vector.bn_aggr(mv[:, c, :], bst[:, c, :])
                dn2 = lnp.tile([128, NC], F32, tag="dn2")
                nc.vector.tensor_mul(dn2, dn, dn)
                vpe = stats.tile([128, NC], F32, tag="vpe")
                nc.vector.scalar_tensor_tensor(
                    out=vpe, in0=dn2, scalar=LN_EPS, in1=mv[:, :, 1],
                    op0=OP.mult, op1=OP.add,
                )
                lnv = stats.tile([128, NC], F32, tag="lnv")
                nc.scalar.activation(lnv, vpe, AF.Ln)
                rstd = stats.tile([128, NC], F32, tag="rstd")
                nc.scalar.activation(rstd, lnv, AF.Exp, scale=-0.5)

                # ---- (h' - mean) * rstd, broadcast ops -------------------------
                hm = lnp.tile([128, NC, D], F32, tag="hm")
                nc.vector.tensor_tensor(
                    out=hm, in0=hp, in1=_bcast_inner(mv[:, :, 0], D),
                    op=OP.subtract,
                )
                o_rows = lnp.tile([128, NC, D], F32, tag="orows")
                nc.vector.tensor_mul(o_rows, hm, _bcast_inner(rstd, D))
                nc.sync.dma_start(
                    out_d[b].rearrange("(c p) d -> p c d", p=128), o_rows
                )
    nc.compile()
    return nc


_CACHED = None


def _get_nc():
    global _CACHED
    if _CACHED is None:
        _CACHED = _build_nc()
    return _CACHED


def _to_bf16(a: np.ndarray) -> np.ndarray:
    if BF16_NP is not None:
        return a.astype(BF16_NP)
    u = np.ascontiguousarray(a.astype(np.float32)).view(np.uint32)
    r = ((u >> 16) & 1).astype(np.uint32)
    return (((u + 0x7FFF + r) >> 16).astype(np.uint16)).view(np.uint16)


def kernel(x: np.ndarray, Wv: np.ndarray, bv: np.ndarray, phi: np.ndarray) -> np.ndarray:
    x = np.ascontiguousarray(np.asarray(x, np.float32))
    Wv = np.asarray(Wv, np.float32)
    bv = np.asarray(bv, np.float32)
    phi = np.asarray(phi, np.float32)
    assert x.shape == (B, S, D), x.shape

    wg = _givens_orthogonal(phi, D)
    wv_aug = np.zeros((DA, DA), np.float32)
    wv_aug[0:D, 0:D] = Wv.T
    wv_aug[D, 0:D] = bv
    wv_aug[D, D] = 1.0

    nc = _get_nc()
    in_maps = [
        {
            "x": np.ascontiguousarray(x[c * NB:(c + 1) * NB]),
            "wg": _to_bf16(wg),
            "wv_aug": _to_bf16(wv_aug),
            "ident_b": _to_bf16(np.eye(128, dtype=np.float32)),
            "ident_f": np.eye(128, dtype=np.float32),
        }
        for c in range(NCORES)
    ]
    res = run_bass_kernel_spmd(nc, in_maps, list(range(NCORES)))
    out = np.concatenate([res.results[c]["out"] for c in range(NCORES)], axis=0)
    return out.astype(np.float32)


if __name__ == "__main__":
    rng = np.random.default_rng(0)
    x = rng.standard_normal((B, S, D)).astype(np.float32)
    Wv = (rng.standard_normal((D, D)) / math.sqrt(D)).astype(np.float32)
    bv = (rng.standard_normal(D) * 0.01).astype(np.float32)
    phi = rng.uniform(0, 2 * math.pi, 2 * D - 3).astype(np.float32)
    y = kernel(x=x, Wv=Wv, bv=bv, phi=phi)
    print("out", y.shape, y.dtype, np.abs(y).mean())
